# revision 44
# baseline (speedup 1.0000x reference)
"""Trainium2 Bass kernel for the dense CNN (pad+border-extrapolate, 4 convs,
pixel shuffle). Data parallel: 2 images per core on 8 cores.

Layouts (per image, spatial maps flat with row stride 266, garbage cols at
row tails are computed but never consumed):
- xp: padded+extrapolated image (272 rows x 266) in DRAM scratch, bf16.
- h1r: (128, S1) SBUF bf16: partition [c | 64+c] = conv1 channel c of
  even|odd rows (interleave mod 2); free = y2*266 + x.
- h2r/h3r: (128, S) bf16: partition [32q + c] = channel c of rows == q mod 4;
  free = y4*266 + x.
- h4b: (128, S4C) bf16: same mod-4 quads, each quad 16 channels + 16 junk.
Convs are tap-accumulated bf16 matmuls into f32 PSUM, col-tiled across PE
column groups (2x for conv1, 4x for conv2/3/4). K-packing uses zero-padded
stacked weight variants so every matmul is a full-K read at rhs base 0.

Pixel shuffle is done ON-CHIP (a 4-elem interleave via DMA would emit 4B
packets -- measured 2.1M packets / 15.7ms wall): 0/1 selection matmuls
route channel (32q+4i+j) to partition (16r+4q+i) for row-octet r, PSUM
holds j-blocks contiguously, the ACT evac interleaves j into 4x+j on the
SBUF write, and the final y DMA moves contiguous 4KB rows.
"""

import numpy as np
import ml_dtypes

import concourse.bass as bass
import concourse.bacc as bacc
import concourse.tile as tile
import concourse.mybir as mybir
from concourse.bass_utils import run_bass_kernel_spmd

F32 = mybir.dt.float32
BF16 = mybir.dt.bfloat16
AF = mybir.ActivationFunctionType
ALU = mybir.AluOpType

W = 266          # flat row stride
XPROWS = 272     # xp rows incl. zero pad rows 266..271
B4 = 16          # output quad-rows (of 4 rows) per band
NBANDS = 4       # B4*4*NBANDS = 256 output rows
B2 = 2 * B4 + 5  # h1r y2-rows per band (37)
NIMG = 2         # images per core
XP = XPROWS * W

S1 = B2 * W                # 9842
S2 = (B4 + 2) * W          # 4788
S3 = (B4 + 1) * W          # 4522
S4 = B4 * W                # 4256
S4C = B4 * 256             # 4096 (conv4 output, no garbage cols)
PAD = 8                    # tile tail pad (halo reads spill a few elements)

# strip layout: per image 6 depth-blocks of 4 edges x 266
# offset(d, e, pos) = d*1064 + e*266 + pos ; e: 0=rowN 1=rowS 2=colW 3=colE
SLEN = 6 * 4 * W  # 6384


def _ap(t, off, dims):
    return bass.AP(tensor=t.tensor, offset=t.offset + off,
                   ap=[list(d) for d in dims])


def build_nc(debug=()):
    nc = bacc.Bacc("TRN2", target_bir_lowering=False)

    xbf = nc.dram_tensor("xbf", [NIMG, 256, 256], BF16, kind="ExternalInput")
    xe = nc.dram_tensor("xe", [NIMG, 4, 256], F32, kind="ExternalInput")
    w1t = nc.dram_tensor("w1t", [25, 64], BF16, kind="ExternalInput")
    w2 = nc.dram_tensor("w2", [128, 12 * 32], BF16, kind="ExternalInput")
    w3 = nc.dram_tensor("w3", [128, 6 * 32], BF16, kind="ExternalInput")
    w4 = nc.dram_tensor("w4", [128, 6 * 16], BF16, kind="ExternalInput")
    selw = nc.dram_tensor("selw", [128, 8 * 32], BF16, kind="ExternalInput")
    b1d = nc.dram_tensor("b1d", [128, 1], F32, kind="ExternalInput")
    b2d = nc.dram_tensor("b2d", [128, 1], F32, kind="ExternalInput")
    b3d = nc.dram_tensor("b3d", [128, 1], F32, kind="ExternalInput")
    b4d = nc.dram_tensor("b4d", [128, 1], F32, kind="ExternalInput")
    y = nc.dram_tensor("y", [NIMG, 1024, 1024], F32, kind="ExternalOutput")
    xp = nc.dram_tensor("xp", [NIMG, XP], BF16,
                        kind="ExternalOutput" if "xp" in debug else "Internal")
    if "h1r" in debug:
        dh1 = nc.dram_tensor("dh1", [128, S1], BF16, kind="ExternalOutput")
    if "h2r" in debug:
        dh2 = nc.dram_tensor("dh2", [128, S2], BF16, kind="ExternalOutput")
    if "h3r" in debug:
        dh3 = nc.dram_tensor("dh3", [128, S3], BF16, kind="ExternalOutput")

    with tile.TileContext(nc) as tc:
        with tc.tile_pool(name="consts", bufs=1) as consts:
            tw1 = consts.tile([25, 64], BF16)
            tw2 = consts.tile([128, 12 * 32], BF16)
            tw3 = consts.tile([128, 6 * 32], BF16)
            tw4 = consts.tile([128, 6 * 16], BF16)
            tsel = consts.tile([128, 8 * 32], BF16)
            tb1 = consts.tile([128, 1], F32)
            tb2 = consts.tile([128, 1], F32)
            tb3 = consts.tile([128, 1], F32)
            tb4 = consts.tile([128, 1], F32)
            for dst, src in ((tw1, w1t), (tw2, w2), (tw3, w3), (tw4, w4),
                             (tsel, selw),
                             (tb1, b1d), (tb2, b2d), (tb3, b3d), (tb4, b4d)):
                nc.scalar.dma_start(out=dst, in_=src[:, :])

            # ------------- border extrapolation (f32 strips) -------------
            with tc.tile_pool(name="strips", bufs=1) as spool:
                st = spool.tile([NIMG, SLEN], F32)
                tmp = spool.tile([NIMG, 4 * W], F32)
                msk = spool.tile([NIMG, 4 * W], mybir.dt.uint8)
                ones = spool.tile([NIMG, 4 * W], F32)
                zpad = spool.tile([NIMG, 6 * W], BF16)
                nc.gpsimd.memset(zpad, 0.0)
                # interior + tail-pad first: independent of the border strips
                for g in range(NIMG):
                    xo = g * XP
                    nc.sync.dma_start(
                        out=_ap(xp[:, :], xo + 5 * W + 5,
                                [[W, 256], [1, 256]]),
                        in_=xbf[g, :, :])
                    nc.sync.dma_start(
                        out=_ap(xp[:, :], xo + 266 * W, [[1, 6 * W]]),
                        in_=zpad[g:g + 1, :])
                # pad value 0.5 is only consumed near ring ends; middles are
                # overwritten by the recurrence/xe loads before any read
                nc.vector.memset(
                    _ap(st, 0, [[SLEN, NIMG], [266, 24], [1, 8]]), 0.5)
                nc.vector.memset(
                    _ap(st, 258, [[SLEN, NIMG], [266, 24], [1, 8]]), 0.5)
                nc.vector.memset(ones, 1.0)

                for k, doff in ((0, 5 * 1064 + 0 + 5), (1, 0 * 1064 + 266 + 5),
                                (2, 5 * 1064 + 532 + 5), (3, 0 * 1064 + 798 + 5)):
                    nc.sync.dma_start(
                        out=_ap(st, doff, [[SLEN, NIMG], [1, 256]]),
                        in_=xe[:, k, :])

                for i in range(5, 0, -1):
                    im = i - 1
                    L = 264 - 2 * i
                    rg = (5 - 2 * i) * 1064 + 266
                    wg = (7 - 2 * i) * 1064 + 266

                    def vin(k):
                        return _ap(st, i * 1064 + i + k,
                                   [[SLEN, NIMG], [rg, 2], [532, 2], [1, L]])
                    tmpa = _ap(tmp, 0, [[4 * W, NIMG], [266, 4], [1, L]])
                    tmpm = _ap(msk, 0, [[4 * W, NIMG], [266, 4], [1, L]])
                    nc.vector.tensor_tensor(out=tmpa, in0=vin(0), in1=vin(1),
                                            op=ALU.add)
                    nc.vector.tensor_tensor(out=tmpa, in0=tmpa, in1=vin(2),
                                            op=ALU.add)
                    nc.vector.tensor_scalar(out=tmpm, in0=tmpa, scalar1=0.9,
                                            scalar2=None, op0=ALU.is_gt)
                    wdst = _ap(st, im * 1064 + i + 1,
                               [[SLEN, NIMG], [wg, 2], [532, 2], [1, L]])
                    nc.vector.tensor_scalar(out=wdst, in0=tmpa,
                                            scalar1=1.0 / 3.0, scalar2=None,
                                            op0=ALU.mult)
                    mview = _ap(msk, 0, [[4 * W, NIMG], [266, 2], [532, 2], [1, L]])
                    oview = _ap(ones, 0, [[4 * W, NIMG], [266, 2], [532, 2], [1, L]])
                    nc.vector.copy_predicated(out=wdst, mask=mview, data=oview)

                    ut = spool.tile([NIMG, 4], F32, tag="ut")

                    def c22(base, grp, pos):
                        return _ap(st, base, [[SLEN, NIMG], [grp, 2], [pos, 2]])
                    utv = _ap(ut, 0, [[4, NIMG], [2, 2], [1, 2]])
                    # u1: (cy,cxp) = ((cyp,cxp) + (cy,cx+2nx))/2
                    nc.vector.tensor_tensor(
                        out=utv,
                        in0=c22(i * 1064 + i, rg, 265 - 2 * i),
                        in1=c22(im * 1064 + i + 1, wg, 263 - 2 * i), op=ALU.add)
                    nc.vector.tensor_scalar(
                        out=c22(im * 1064 + i, wg, 265 - 2 * i), in0=utv,
                        scalar1=0.5, scalar2=None, op0=ALU.mult)
                    nc.vector.tensor_scalar(
                        out=c22(i * 1064 + 532 + im, rg, 267 - 2 * i), in0=utv,
                        scalar1=0.5, scalar2=None, op0=ALU.mult)
                    # u2: (cyp,cx) = ((cyp,cxp) + (cy+2ny,cx))/2
                    nc.vector.tensor_tensor(
                        out=utv,
                        in0=c22(i * 1064 + i, rg, 265 - 2 * i),
                        in1=c22(im * 1064 + 532 + i + 1, wg, 263 - 2 * i),
                        op=ALU.add)
                    nc.vector.tensor_scalar(
                        out=c22(i * 1064 + im, rg, 267 - 2 * i), in0=utv,
                        scalar1=0.5, scalar2=None, op0=ALU.mult)
                    nc.vector.tensor_scalar(
                        out=c22(im * 1064 + 532 + i, wg, 265 - 2 * i), in0=utv,
                        scalar1=0.5, scalar2=None, op0=ALU.mult)
                    # u3: (cy,cx) = ((cy,cxp) + (cyp,cx))/2
                    nc.vector.tensor_tensor(
                        out=utv,
                        in0=c22(im * 1064 + i, wg, 265 - 2 * i),
                        in1=c22(i * 1064 + im, rg, 267 - 2 * i), op=ALU.add)
                    nc.vector.tensor_scalar(
                        out=c22(im * 1064 + im, wg, 267 - 2 * i), in0=utv,
                        scalar1=0.5, scalar2=None, op0=ALU.mult)
                    nc.vector.tensor_scalar(
                        out=c22(im * 1064 + 532 + im, wg, 267 - 2 * i), in0=utv,
                        scalar1=0.5, scalar2=None, op0=ALU.mult)

                # cast to bf16, reordering into contiguous assembly blocks:
                # [rowN d-major 1596][rowS 1596][colW y-major 1596][colE 1596]
                stb = spool.tile([NIMG, SLEN], BF16)
                nc.gpsimd.tensor_copy(
                    out=_ap(stb, 2 * 1596, [[SLEN, NIMG], [6, 266], [1, 6]]),
                    in_=_ap(st, 532, [[SLEN, NIMG], [1, 266], [1064, 6]]))
                nc.scalar.activation(
                    out=_ap(stb, 3 * 1596, [[SLEN, NIMG], [6, 266], [1, 6]]),
                    in_=_ap(st, 798, [[SLEN, NIMG], [1, 266], [1064, 6]]),
                    func=AF.Copy, scale=1.0)
                nc.vector.tensor_copy(
                    out=_ap(stb, 0, [[SLEN, NIMG], [266, 6], [1, 266]]),
                    in_=_ap(st, 0, [[SLEN, NIMG], [1064, 6], [1, 266]]))
                nc.gpsimd.tensor_copy(
                    out=_ap(stb, 1596, [[SLEN, NIMG], [266, 6], [1, 266]]),
                    in_=_ap(st, 266, [[SLEN, NIMG], [1064, 6], [1, 266]]))
                # rows own rows 0..5 / 260..265 outright; col strips only
                # write rows 6..259 so the four DMAs are disjoint (they run
                # on different queues, so overlap order would be undefined)
                for g in range(NIMG):
                    xo = g * XP
                    so = g * SLEN
                    nc.sync.dma_start(
                        out=_ap(xp[:, :], xo + 6 * W, [[W, 254], [1, 6]]),
                        in_=_ap(stb, so + 2 * 1596 + 36,
                                [[SLEN, 1], [1, 254 * 6]]))
                    nc.gpsimd.dma_start(
                        out=_ap(xp[:, :], xo + 6 * W + 260,
                                [[W, 254], [1, 6]]),
                        in_=_ap(stb, so + 3 * 1596 + 36,
                                [[SLEN, 1], [1, 254 * 6]]))
                    nc.scalar.dma_start(
                        out=_ap(xp[:, :], xo + 0, [[1, 1596]]),
                        in_=_ap(stb, so + 0, [[SLEN, 1], [1, 1596]]))
                    nc.gpsimd.dma_start(
                        out=_ap(xp[:, :], xo + 260 * W, [[1, 1596]]),
                        in_=_ap(stb, so + 1596, [[SLEN, 1], [1, 1596]]))

            # --------------- conv pipeline ---------------
            with tc.tile_pool(name="t2col", bufs=2) as tpool, \
                 tc.tile_pool(name="hmaps", bufs=1) as hpool, \
                 tc.tile_pool(name="h4", bufs=2) as h4pool, \
                 tc.tile_pool(name="ps", bufs=8, space="PSUM") as pspool:

                def load_t2col(img, band, first=False):
                    xrow0 = 4 * band * B4
                    tev = tpool.tile([25, S1], BF16, tag="tev")
                    tod = tpool.tile([25, S1], BF16, tag="tod")
                    ev_eng = nc.scalar if first else nc.sync
                    for ky in range(5):
                        ev_eng.dma_start(
                            out=_ap(tev, ky * 5 * S1, [[S1, 5], [1, S1]]),
                            in_=_ap(xp[:, :], img * XP + (xrow0 + ky) * W,
                                    [[1, 5], [2 * W, B2], [1, W]]))
                        nc.sync.dma_start(
                            out=_ap(tod, ky * 5 * S1, [[S1, 5], [1, S1]]),
                            in_=_ap(xp[:, :],
                                    img * XP + (xrow0 + 1 + ky) * W,
                                    [[1, 5], [2 * W, B2], [1, W]]))
                    return tev, tod

                def emit_conv1(tev, tod):
                    # conv1: 1->64, K=25, two col-tiles even/odd
                    h1r = hpool.tile([128, S1 + PAD], BF16, tag="h1r")
                    flip = 0
                    for j in range(0, S1, 512):
                        n = min(512, S1 - j)
                        ps = pspool.tile([128, 512], F32, tag="ps")
                        nc.tensor.matmul(ps[0:64, 0:n], tw1[:, :],
                                         tev[:, j:j + n], start=True,
                                         stop=True, tile_position=(0, 0))
                        nc.tensor.matmul(ps[64:128, 0:n], tw1[:, :],
                                         tod[:, j:j + n], start=True,
                                         stop=True, tile_position=(0, 64))
                        if flip % 2 == 0:
                            nc.scalar.activation(
                                out=h1r[:, j:j + n], in_=ps[:, 0:n],
                                func=AF.Relu, bias=tb1[:, :], scale=1.0)
                        else:
                            nc.vector.tensor_scalar(
                                out=h1r[:, j:j + n], in0=ps[:, 0:n],
                                scalar1=tb1[:, :], scalar2=0.0,
                                op0=ALU.add, op1=ALU.max)
                        flip += 1
                    return h1r

                seq = [(ig, bd) for ig in range(NIMG)
                       for bd in range(NBANDS)]
                # band b+1's conv1 is emitted in band b's tail (see below);
                # h1r_pend carries the finished tile into the next iteration
                tt = load_t2col(*seq[0], first=True)
                h1r_pend = emit_conv1(*tt)
                if len(seq) > 1:
                    tt = load_t2col(*seq[1])
                for bi, (img, band) in enumerate(seq):
                    if True:
                        gq = band * B4
                        h1r = h1r_pend

                        h2r = hpool.tile([128, S2 + PAD], BF16, tag="h2r")
                        h2rB = hpool.tile([128, S2 + PAD], BF16, tag="h2rB")
                        h3r = hpool.tile([128, S3 + PAD], BF16, tag="h3r")
                        h3rB = hpool.tile([128, S3 + PAD], BF16, tag="h3rB")
                        h4b = h4pool.tile([128, S4C + 16], BF16, tag="h4b")
                        ycont = h4pool.tile([128, 2048], F32, tag="ycont")

                        # ---- conv2: 64->32, 2-row x 256 groups + 10-col tail --
                        # tail pass first so the incremental h2rB row copies
                        # (issued right after each A-group evac) see final rows
                        NR2 = B4 + 2
                        psb = pspool.tile([128, 512], F32, tag="ps")
                        for c in range(4):
                            first = True
                            for kx in range(3):
                                for dd in (0, 1):
                                    v = (c % 2) * 2 + dd
                                    lhs = tw2[:, (kx * 4 + v) * 32:
                                              (kx * 4 + v) * 32 + 32]
                                    roff = ((c // 2) + dd) * W + kx + 256
                                    nc.tensor.matmul(
                                        _ap(psb, 32 * c * 512,
                                            [[512, 32], [10, NR2], [1, 10]]),
                                        lhs,
                                        _ap(h1r, roff,
                                            [[S1 + PAD, 128], [532, NR2],
                                             [1, 10]]),
                                        start=first,
                                        stop=(kx == 2 and dd == 1),
                                        tile_position=(0, 32 * c),
                                        skip_group_check=True)
                                    first = False
                        nc.vector.tensor_scalar(
                            out=_ap(h2r, 256, [[S2 + PAD, 128], [W, NR2],
                                               [1, 10]]),
                            in0=_ap(psb, 0, [[512, 128], [10, NR2], [1, 10]]),
                            scalar1=tb2[:, :], scalar2=0.0, op0=ALU.add,
                            op1=ALU.max)
                        flip2 = 0
                        for y4l in range(0, NR2, 2):
                            psa = pspool.tile([128, 512], F32, tag="ps")
                            for c in range(4):
                                first = True
                                for kx in range(3):
                                    for dd in (0, 1):
                                        v = (c % 2) * 2 + dd
                                        lhs = tw2[:, (kx * 4 + v) * 32:
                                                  (kx * 4 + v) * 32 + 32]
                                        roff = ((2 * y4l + (c // 2) + dd) * W
                                                + kx)
                                        nc.tensor.matmul(
                                            _ap(psa, 32 * c * 512,
                                                [[512, 32], [256, 2], [1, 256]]),
                                            lhs,
                                            _ap(h1r, roff,
                                                [[S1 + PAD, 128], [532, 2],
                                                 [1, 256]]),
                                            start=first,
                                            stop=(kx == 2 and dd == 1),
                                            tile_position=(0, 32 * c),
                                            skip_group_check=True)
                                        first = False
                            dst = _ap(h2r, y4l * W,
                                      [[S2 + PAD, 128], [W, 2], [1, 256]])
                            pin = _ap(psa, 0, [[512, 128], [256, 2], [1, 256]])
                            if flip2 % 2 == 0:
                                nc.scalar.activation(out=dst, in_=pin,
                                                     func=AF.Relu,
                                                     bias=tb2[:, :], scale=1.0)
                            else:
                                nc.vector.tensor_scalar(out=dst, in0=pin,
                                                        scalar1=tb2[:, :],
                                                        scalar2=0.0,
                                                        op0=ALU.add,
                                                        op1=ALU.max)
                            flip2 += 1
                            # incremental shifted copy of finished rows:
                            # h2rB blocks {0,1} = h2 classes {2,3}; blocks
                            # {2,3} = classes {0,1} of the next quad (-W).
                            nc.gpsimd.dma_start(
                                out=h2rB[0:64, y4l * W:(y4l + 2) * W],
                                in_=h2r[64:128, y4l * W:(y4l + 2) * W])
                            if y4l == 0:
                                nc.gpsimd.dma_start(
                                    out=h2rB[64:128, 0:W],
                                    in_=h2r[0:64, W:2 * W])
                            else:
                                nc.gpsimd.dma_start(
                                    out=h2rB[64:128,
                                             (y4l - 1) * W:(y4l + 1) * W],
                                    in_=h2r[0:64, y4l * W:(y4l + 2) * W])

                        # ---- conv3: 3 taps per lane (lanes 2,3 on h2rB) ----
                        fl = 0
                        for j in range(0, S3, 512):
                            n = min(512, S3 - j)
                            ps = pspool.tile([128, 512], F32, tag="ps")
                            for c in range(4):
                                src = h2r if c < 2 else h2rB
                                v = c % 2
                                for kx in range(3):
                                    lhs = tw3[:, (kx * 2 + v) * 32:
                                              (kx * 2 + v) * 32 + 32]
                                    nc.tensor.matmul(
                                        _ap(ps, 32 * c * 512,
                                            [[512, 32], [1, n]]),
                                        lhs,
                                        _ap(src, j + kx,
                                            [[S2 + PAD, 128], [1, n]]),
                                        start=(kx == 0), stop=(kx == 2),
                                        tile_position=(0, 32 * c),
                                        skip_group_check=True)
                            if fl % 2 == 0:
                                nc.scalar.activation(
                                    out=h3r[:, j:j + n], in_=ps[:, 0:n],
                                    func=AF.Relu, bias=tb3[:, :], scale=1.0)
                            else:
                                nc.vector.tensor_scalar(
                                    out=h3r[:, j:j + n], in0=ps[:, 0:n],
                                    scalar1=tb3[:, :], scalar2=0.0,
                                    op0=ALU.add, op1=ALU.max)
                            fl += 1
                            # incremental shifted copy of the finished chunk
                            nc.gpsimd.dma_start(
                                out=h3rB[0:64, j:j + n],
                                in_=h3r[64:128, j:j + n])
                            if j == 0:
                                nc.gpsimd.dma_start(
                                    out=h3rB[64:128, 0:n - W],
                                    in_=h3r[0:64, W:n])
                            else:
                                nc.gpsimd.dma_start(
                                    out=h3rB[64:128, j - W:j + n - W],
                                    in_=h3r[0:64, j:j + n])

                        # ---- conv4 + pixel shuffle, interleaved by y4 octet
                        # so the perm matmuls chase the sigmoid evacs ----
                        for h in (0, 1):
                            for y4l in range(8 * h, 8 * h + 8, 2):
                                psa = pspool.tile([128, 512], F32, tag="ps")
                                for c in range(4):
                                    src = h3r if c < 2 else h3rB
                                    v = c % 2
                                    for kx in range(3):
                                        lhs = tw4[:, (kx * 2 + v) * 16:
                                                  (kx * 2 + v) * 16 + 16]
                                        nc.tensor.matmul(
                                            _ap(psa, 32 * c * 512,
                                                [[512, 16], [256, 2],
                                                 [1, 256]]),
                                            lhs,
                                            _ap(src, y4l * W + kx,
                                                [[S3 + PAD, 128], [W, 2],
                                                 [1, 256]]),
                                            start=(kx == 0), stop=(kx == 2),
                                            tile_position=(0, 32 * c),
                                            skip_group_check=True)
                                nc.scalar.activation(
                                    out=_ap(h4b, y4l * 256,
                                            [[S4C + 16, 128], [256, 2],
                                             [1, 256]]),
                                    in_=_ap(psa, 0,
                                            [[512, 128], [256, 2], [1, 256]]),
                                    func=AF.Sigmoid, bias=tb4[:, :],
                                    scale=1.0)

                            if h == 1 and bi + 1 < len(seq):
                                # fill the tail bubble (perm waits on ACT
                                # sigmoids) with the next band's conv1
                                h1r_pend = emit_conv1(*tt)
                                if bi + 2 < len(seq):
                                    tt = load_t2col(*seq[bi + 2])

                            # perm for this octet: psY[32g+16e+4q+i, j*128+x]
                            # = h4b[32q+4i+j, y4*256+xc*128+x], y4 = 8h+2g+e;
                            # evac interleaves j: ycont[p, h*1024+xc*512+4x+j]
                            for xc in (0, 1):
                                ps = pspool.tile([128, 512], F32, tag="ps")
                                for g in range(4):
                                    for e in (0, 1):
                                        y4 = 8 * h + 2 * g + e
                                        for j in range(4):
                                            # e=0/e=1 accumulate: each writes
                                            # zeros in the other's 16-row half
                                            sel = tsel[:, (e * 4 + j) * 32:
                                                       (e * 4 + j) * 32 + 32]
                                            nc.tensor.matmul(
                                                _ap(ps, 32 * g * 512 + j * 128,
                                                    [[512, 32], [1, 128]]),
                                                sel,
                                                _ap(h4b,
                                                    y4 * 256 + xc * 128,
                                                    [[S4C + 16, 128], [1, 128]]),
                                                start=(e == 0 and j == 0),
                                                stop=(e == 1 and j == 3),
                                                tile_position=(0, 32 * g),
                                                skip_group_check=True)
                                if xc == 0:
                                    nc.scalar.activation(
                                        out=_ap(ycont, h * 1024 + xc * 512,
                                                [[2048, 128], [1, 4],
                                                 [4, 128]]),
                                        in_=_ap(ps, 0,
                                                [[512, 128], [128, 4],
                                                 [1, 128]]),
                                        func=AF.Copy, scale=1.0)
                                else:
                                    nc.vector.tensor_copy(
                                        out=_ap(ycont, h * 1024 + xc * 512,
                                                [[2048, 128], [1, 4],
                                                 [4, 128]]),
                                        in_=_ap(ps, 0,
                                                [[512, 128], [128, 4],
                                                 [1, 128]]))

                        # y[img, 16*(gq+y4) + 4q+i, X] = ycont[16r+4q+i,
                        #   h*1024+X] for y4 = 8h+r; contiguous 4KB rows.
                        for q in range(4):
                            for i in range(4):
                                nc.sync.dma_start(
                                    out=_ap(y[:, :, :],
                                            img * 1024 * 1024
                                            + (16 * gq + 4 * q + i) * 1024,
                                            [[16 * 1024, 8], [128 * 1024, 2],
                                             [1, 1024]]),
                                    in_=_ap(ycont, (4 * q + i) * 2048,
                                            [[16 * 2048, 8], [1, 2048]]))

                        if "h1r" in debug and img == 0 and band == 0:
                            nc.sync.dma_start(out=dh1[:, :],
                                              in_=h1r[:, 0:S1])
                        if "h2r" in debug and img == 0 and band == 0:
                            nc.sync.dma_start(out=dh2[:, :],
                                              in_=h2r[:, 0:S2])
                        if "h3r" in debug and img == 0 and band == 0:
                            nc.sync.dma_start(out=dh3[:, :],
                                              in_=h3r[:, 0:S3])

    nc.finalize()
    return nc


def host_inputs(x, W1, b1, W2, b2, W3, b3, W4, b4, core):
    """Build the per-core input map (images 2*core, 2*core+1)."""
    xi = np.asarray(x[2 * core:2 * core + 2], dtype=np.float32)
    bf = ml_dtypes.bfloat16

    xe = np.stack([xi[:, 0, :], xi[:, 255, :], xi[:, :, 0], xi[:, :, 255]],
                  axis=1).astype(np.float32)

    w1t = np.ascontiguousarray(np.asarray(W1)[:, 0].reshape(64, 25).T)

    w2v = np.zeros((128, 12 * 32), np.float32)
    for kx in range(3):
        Wk = [np.asarray(W2)[:, :, ky, kx].T for ky in range(3)]  # (64,32)
        Z = np.zeros_like(Wk[0])
        var = [np.concatenate([Wk[0], Wk[1]], 0),
               np.concatenate([Wk[2], Z], 0),
               np.concatenate([Z, Wk[0]], 0),
               np.concatenate([Wk[1], Wk[2]], 0)]
        for v in range(4):
            w2v[:, (kx * 4 + v) * 32:(kx * 4 + v) * 32 + 32] = var[v]

    def conv34_vars(Wc, M):
        w = np.zeros((128, 6 * M), np.float32)
        for kx in range(3):
            Wk = [np.asarray(Wc)[:, :, ky, kx].T for ky in range(3)]  # (32,M)
            Z = np.zeros_like(Wk[0])
            var = [np.concatenate([Wk[0], Wk[1], Wk[2], Z], 0),
                   np.concatenate([Z, Wk[0], Wk[1], Wk[2]], 0)]
            for v in range(2):
                w[:, (kx * 2 + v) * M:(kx * 2 + v) * M + M] = var[v]
        return w

    w3v = conv34_vars(W3, 32)
    w4v = conv34_vars(W4, 16)

    # pixel-shuffle selection matrices: S[e][j][k=32q+4i+j, m=16e+4q+i] = 1
    selw = np.zeros((128, 8 * 32), np.float32)
    for e in range(2):
        for j in range(4):
            for q in range(4):
                for i in range(4):
                    selw[32 * q + 4 * i + j,
                         (e * 4 + j) * 32 + 16 * e + 4 * q + i] = 1.0

    b1x = np.concatenate([b1, b1]).reshape(128, 1).astype(np.float32)
    b2x = np.tile(b2, 4).reshape(128, 1).astype(np.float32)
    b3x = np.tile(b3, 4).reshape(128, 1).astype(np.float32)
    b4x = np.zeros((128, 1), np.float32)
    for q in range(4):
        b4x[32 * q:32 * q + 16, 0] = b4

    return {
        "xbf": xi.astype(bf),
        "xe": xe,
        "w1t": w1t.astype(bf),
        "w2": w2v.astype(bf),
        "w3": w3v.astype(bf),
        "w4": w4v.astype(bf),
        "selw": selw.astype(bf),
        "b1d": b1x, "b2d": b2x, "b3d": b3x, "b4d": b4x,
    }


_NC_CACHE = {}

LAST_EXEC_NS = None


def _get_nc(debug=()):
    key = tuple(sorted(debug))
    if key not in _NC_CACHE:
        _NC_CACHE[key] = build_nc(debug)
    return _NC_CACHE[key]


def kernel(x, W1, b1, W2, b2, W3, b3, W4, b4, _debug=(), _results=None):
    global LAST_EXEC_NS
    nc = _get_nc(_debug)
    in_maps = [host_inputs(x, W1, b1, W2, b2, W3, b3, W4, b4, core)
               for core in range(8)]
    import os
    tmpdir = os.environ.get("BASS_TMPDIR") or None
    if tmpdir:
        os.makedirs(tmpdir, exist_ok=True)
    res = run_bass_kernel_spmd(nc, in_maps, core_ids=list(range(8)),
                               tmpdir=tmpdir)
    LAST_EXEC_NS = res.exec_time_ns
    if _results is not None:
        _results.extend(res.results)
    out = np.concatenate([r["y"] for r in res.results], axis=0)
    return np.ascontiguousarray(out.astype(np.float32))

